# revision 16
# baseline (speedup 1.0000x reference)
"""Trainium2 Bass kernel for nn_DifferentiableDAG (single DAG forward step).

Strategy
--------
The reference materializes a (B, T, NT, H) gather `ge` of node_embeds and runs
attention over the flattened per-step node sets.  But the gather is just a
re-indexing: flat position i at step t maps to (n, tau) = (i//(t+1), i%(t+1)),
and softmax is permutation invariant.  So everything can stay in the natural
m = n*T + tau layout:

  K      = node_embeds[b] @ Wk + bk                      (512, 512)
  S      = q @ K^T                                       (Tloc, 512)
  w      = A*clip(S, +-40) + Bc    (A, Bc host-precomputed masks that fold in
                                    the masked_fill -> -40 path, the /TAU and
                                    the *pre-permuted* Gumbel noise)
  att    = softmax(w) + 1e-10*valid
  e/v    = att @ [node_embeds[b] | node_values[b]]
  ...ops/op_w/MLP as in the reference.

The Gumbel noise is input-independent (key 42), computed on host CPU with the
same jax PRNG and permuted into the m layout.  The 0.5*(e1+e2) residual is
folded into Wmlp rows [0:1024] (+0.5*I) on the host.

Sharding: 8 cores = (4 batches) x (2 halves of T=64).  Embarrassingly
parallel; all params replicated; no collectives.
"""

import os
import sys

import numpy as np

for _p in ("/opt/trn_rl_repo", "/root/.axon_site/_ro/trn_rl_repo"):
    if os.path.isdir(_p) and _p not in sys.path:
        sys.path.append(_p)

import concourse.bass as bass
import concourse.mybir as mybir
from concourse import tile
from concourse.bass_utils import run_bass_kernel_spmd

F32 = mybir.dt.float32
AF = mybir.ActivationFunctionType
ALU = mybir.AluOpType
AX = mybir.AxisListType

B, N, T, H = 4, 8, 64, 512
NOPS = 9
NT = N * T          # 512 flattened (n, tau) positions
TLOC = 32           # timesteps per core
NCORES = 8
EPS = 1e-8
INVALID_FILL = -55.0  # ~-inf for softmax; keeps ACT Exp input inside table range

_CACHE = {}


# --------------------------------------------------------------------------
# Host-side constants (input independent)
# --------------------------------------------------------------------------

def _gumbel():
    """Gumbel noise exactly as the reference draws it.

    NOTE: the default PRNG impl in this environment is `rbg`, whose bits are
    backend-dependent.  The graded reference runs on the default backend, so
    we must draw on the default backend too (no device override).
    """
    if "gumbel" in _CACHE:
        return _CACHE["gumbel"]
    import jax
    import jax.numpy as jnp

    key = jax.random.key(42)
    k1, k2, k3 = jax.random.split(key, 3)
    g1 = np.asarray(jax.random.gumbel(k1, (B, T, NT), jnp.float32))
    g2 = np.asarray(jax.random.gumbel(k2, (B, T, NT), jnp.float32))
    g3 = np.asarray(jax.random.gumbel(k3, (B, T, NOPS), jnp.float32))
    _CACHE["gumbel"] = (g1, g2, g3)
    return _CACHE["gumbel"]


def _mask_tables():
    """Per-(t, m) layout tables. m = n*T + tau."""
    if "tables" in _CACHE:
        return _CACHE["tables"]
    t = np.arange(T)[:, None]                  # (T, 1)
    m = np.arange(NT)[None, :]
    n = m // T
    tau = m % T
    valid = tau <= t                           # (T, NT)
    flat_i = n * (t + 1) + tau                 # <= 511 always, safe gather index
    masked = (flat_i % T) > t                  # reference's (i % T) <= t mask
    A = np.where(valid & ~masked, np.float32(0.5), np.float32(0.0)).astype(np.float32)
    vadd = np.where(valid, np.float32(1e-10), np.float32(0.0)).astype(np.float32)
    _CACHE["tables"] = (valid, flat_i, masked, A, vadd)
    return _CACHE["tables"]


def _b_const(g, b):
    """B-constant per (t, m) given full gumbel g (B, T, NT): w = A*clip(S) + Bc."""
    valid, flat_i, masked, _, _ = _mask_tables()
    # permute gumbel from flat-i layout into m layout (all fp32 ops, exact)
    gmat = g[b, np.arange(T)[:, None], flat_i]                 # (T, NT) f32
    base = np.where(masked, (np.float32(-40.0) + gmat), gmat)  # f32 add, exact
    return np.where(valid, base * np.float32(0.5),
                    np.float32(INVALID_FILL)).astype(np.float32)


def _t128(v):
    """(512,) -> (128, 4) with v[c*128+p] at [p, c] (per-partition chunks)."""
    return np.ascontiguousarray(v.reshape(4, 128).T).astype(np.float32)


# --------------------------------------------------------------------------
# Bass program (identical on all 8 cores; data differs per core)
# --------------------------------------------------------------------------

def _patch_drain_split():
    """Walrus on this target accepts at most ONE sync-wait per instruction.

    Tile's kernel-tail drain carries one wait per outstanding proc (11 for
    this kernel), which the codegen rejects.  Emit one single-wait drain per
    proc instead — identical semantics, one wait each.
    """
    if getattr(tile.TileContext, "_drain_split_patched", False):
        return
    from concourse.vector_clock import ScopedClock, VectorClock

    def _split(self, tick_clock, wait_clock):
        nc = self.nc
        ticks = eval(repr(tick_clock.global_clock)
                     .replace("VectorClock(", "").rstrip(")"))
        for p, t in enumerate(ticks):
            if t <= 0:
                continue
            vc = VectorClock()
            vc.require_at_least(p, t)
            d = nc.sync.drain()
            wait_clock.add_sem_waits(d.ins, ScopedClock({None: vc}))
        nc.all_engine_barrier()
        popped = nc._tile_sem_poison_stack.pop()
        assert popped is self._sem_poison
        nc.clear_and_free_semaphores(list(self.sems.allocated().values()))
        nc.all_engine_barrier()

    tile.TileContext._drain_and_barrier = _split
    tile.TileContext._drain_split_patched = True


def _build_nc():
    if "nc" in _CACHE:
        return _CACHE["nc"]
    _patch_drain_split()
    nc = bass.Bass()

    def inp(name, shape):
        return nc.dram_tensor(name, list(shape), F32, kind="ExternalInput")

    ne = inp("ne", (NT, H))
    nvp = inp("nvp", (128, 4))
    octx = inp("octx", (TLOC, H))
    opctx = inp("opctx", (TLOC, H))
    s0p = inp("s0p", (128, 4))
    wq1 = inp("wq1", (H, H))
    wq2 = inp("wq2", (H, H))
    wopq = inp("wopq", (H, H))
    wk = inp("wk", (H, H))
    bq1t = inp("bq1t", (128, 4))
    bq2t = inp("bq2t", (128, 4))
    bopqt = inp("bopqt", (128, 4))
    bkt = inp("bkt", (128, 4))
    wsel = inp("wsel", (H, NOPS))
    bsel = inp("bsel", (1, NOPS))
    wse = inp("wse", (1, H))
    bse = inp("bse", (1, H))
    wmlp = inp("wmlp", (4 * H + NOPS, H))
    bmlp = inp("bmlp", (1, H))
    a2 = inp("a2", (2 * TLOC, NT))
    b2 = inp("b2", (2 * TLOC, NT))
    vad = inp("vad", (2 * TLOC, NT))
    g3h = inp("g3h", (TLOC, NOPS))
    ident = inp("ident", (128, 128))
    ones = inp("ones", (1, 64))

    oemb = nc.dram_tensor("oemb", [TLOC, H], F32, kind="ExternalOutput")
    oval = nc.dram_tensor("oval", [TLOC, 1], F32, kind="ExternalOutput")

    with tile.TileContext(nc) as tc:
        with (
            tc.tile_pool(name="sb", bufs=1) as sb,
            tc.tile_pool(name="ps_tr", bufs=2, space="PSUM") as ps_tr,
            tc.tile_pool(name="ps_big", bufs=2, space="PSUM") as ps_big,
            tc.tile_pool(name="ps_sm", bufs=2, space="PSUM") as ps_sm,
            tc.tile_pool(name="ps_gate", bufs=1, space="PSUM") as ps_gate,
            tc.tile_pool(name="ps_mlp", bufs=1, space="PSUM") as ps_mlp,
        ):
            # ---------------- loads ----------------
            def load(dram, shape, pattern=None, tag=None):
                tl = sb.tile(list(shape), F32, tag=tag or dram.name)
                src = dram[:]
                if pattern is not None:
                    src = src.rearrange(pattern, p=128)
                nc.sync.dma_start(out=tl[:], in_=src)
                return tl

            ne_sb = load(ne, (128, 4, H), "(c p) h -> p c h")
            wq1_sb = load(wq1, (128, 4, H), "(c p) h -> p c h")
            wq2_sb = load(wq2, (128, 4, H), "(c p) h -> p c h")
            wopq_sb = load(wopq, (128, 4, H), "(c p) h -> p c h")
            wk_sb = load(wk, (128, 4, H), "(c p) h -> p c h")
            wsel_sb = load(wsel, (128, 4, NOPS), "(c p) j -> p c j")
            wmlp_sb = sb.tile([128, 16, H], F32, tag="wmlp")
            nc.sync.dma_start(
                out=wmlp_sb[:],
                in_=wmlp[0:2048, :].rearrange("(c p) h -> p c h", p=128),
            )
            wmlpt_sb = sb.tile([NOPS, H], F32, tag="wmlpt")
            nc.sync.dma_start(out=wmlpt_sb[:], in_=wmlp[2048:2057, :])

            nvp_sb = load(nvp, (128, 4))
            octx_sb = load(octx, (TLOC, H))
            opctx_sb = load(opctx, (TLOC, H))
            s0p_sb = load(s0p, (128, 4))
            bq1t_sb = load(bq1t, (128, 4))
            bq2t_sb = load(bq2t, (128, 4))
            bopqt_sb = load(bopqt, (128, 4))
            bkt_sb = load(bkt, (128, 4))
            bsel_sb = load(bsel, (1, NOPS))
            wse_sb = load(wse, (1, H))
            bse_sb = load(bse, (1, H))
            bmlp_sb = load(bmlp, (1, H))
            a2_sb = load(a2, (2 * TLOC, NT))
            b2_sb = load(b2, (2 * TLOC, NT))
            vad_sb = load(vad, (2 * TLOC, NT))
            g3h_sb = load(g3h, (TLOC, NOPS))
            id_sb = load(ident, (128, 128))
            ones_sb = load(ones, (1, 64))

            # Walrus on this target rejects engine instructions carrying more
            # than one sync-wait.  Discipline used below:
            #  * every PSUM->SBUF copy / bias-add runs on DVE, so slot-WAR
            #    releases and data producers seen by PE are all DVE;
            #  * ACT runs only transcendentals whose operands are DVE tiles;
            #  * before an engine first reads a DMA-loaded tile, a tiny "gate"
            #    op on that engine waits for that DMA alone, so real compute
            #    instructions never pair a DMA wait with a compute wait.
            gate_ps = ps_gate.tile([1, 32], F32, tag="gate")
            pe_gated = [0]

            def pe_gate(*aps):
                for ap in aps:
                    i = pe_gated[0]
                    pe_gated[0] = i + 1
                    nc.tensor.transpose(
                        gate_ps[0:1, i:i + 1], ap, id_sb[0:1, 0:1]
                    )

            dve_gate_sb = sb.tile([1, 32], F32, tag="dveg")
            dve_gated = [0]

            def dve_gate(*aps):
                for ap in aps:
                    i = dve_gated[0]
                    dve_gated[0] = i + 1
                    nc.vector.tensor_copy(dve_gate_sb[0:1, i:i + 1], ap)

            # ---------------- ne^T (for K^T) ----------------
            pe_gate(id_sb[0:1, 0:1], ne_sb[0:1, 0, 0:1])
            nft_sb = sb.tile([128, 4, NT], F32, tag="nft")  # [h%128, hc, m]
            for mc in range(4):
                for hc in range(4):
                    tr = ps_tr.tile([128, 128], F32, tag="tr")
                    nc.tensor.transpose(
                        tr[:], ne_sb[:, mc, hc * 128:(hc + 1) * 128], id_sb[:]
                    )
                    nc.vector.tensor_copy(
                        nft_sb[:, hc, mc * 128:(mc + 1) * 128], tr[:]
                    )

            # ---------------- K^T = (ne @ Wk + bk)^T ----------------
            pe_gate(wk_sb[0:1, 0, 0:1])
            dve_gate(bkt_sb[0:1, 0:1])
            kt_sb = sb.tile([128, 4, NT], F32, tag="kt")  # [h_out%128, hoc, m]
            for hoc in range(4):
                kp = ps_big.tile([128, NT], F32, tag="big")
                for ci in range(4):
                    nc.tensor.matmul(
                        kp[:],
                        wk_sb[:, ci, hoc * 128:(hoc + 1) * 128],
                        nft_sb[:, ci, :],
                        start=(ci == 0),
                        stop=(ci == 3),
                    )
                nc.vector.tensor_scalar(
                    kt_sb[:, hoc, :], kp[:], bkt_sb[:, hoc:hoc + 1], None, ALU.add
                )

            # ---------------- ctx^T (+ step0) ----------------
            pe_gate(octx_sb[0:1, 0:1], opctx_sb[0:1, 0:1])
            dve_gate(s0p_sb[0:1, 0:1])

            def ctx_T(src_sb, dst_tag):
                dst = sb.tile([128, 4, TLOC], F32, tag=dst_tag)
                for c in range(4):
                    tr = ps_tr.tile([128, TLOC], F32, tag="tr")
                    nc.tensor.transpose(
                        tr[:], src_sb[:, c * 128:(c + 1) * 128], id_sb[:TLOC, :TLOC]
                    )
                    nc.vector.tensor_scalar(
                        dst[:, c, :], tr[:], s0p_sb[:, c:c + 1], None, ALU.add
                    )
                return dst

            octxT_sb = ctx_T(octx_sb, "octxT")
            opctxT_sb = ctx_T(opctx_sb, "opctxT")

            # ---------------- q1^T | q2^T (stacked) and opq^T ----------------
            pe_gate(wq1_sb[0:1, 0, 0:1], wq2_sb[0:1, 0, 0:1],
                    wopq_sb[0:1, 0, 0:1])
            dve_gate(bq1t_sb[0:1, 0:1], bq2t_sb[0:1, 0:1], bopqt_sb[0:1, 0:1])
            q12t_sb = sb.tile([128, 4, 2 * TLOC], F32, tag="q12t")
            opqt_sb = sb.tile([128, 4, TLOC], F32, tag="opqt")

            def qT(w_sb, bt_sb, srcT, dst, off):
                for co in range(4):
                    qp = ps_sm.tile([128, TLOC], F32, tag="sm")
                    for ci in range(4):
                        nc.tensor.matmul(
                            qp[:],
                            w_sb[:, ci, co * 128:(co + 1) * 128],
                            srcT[:, ci, :],
                            start=(ci == 0),
                            stop=(ci == 3),
                        )
                    nc.vector.tensor_scalar(
                        dst[:, co, off:off + TLOC], qp[:], bt_sb[:, co:co + 1],
                        None, ALU.add,
                    )

            qT(wq1_sb, bq1t_sb, octxT_sb, q12t_sb, 0)
            qT(wq2_sb, bq2t_sb, octxT_sb, q12t_sb, TLOC)
            qT(wopq_sb, bopqt_sb, opctxT_sb, opqt_sb, 0)

            # ---------------- S = q @ K^T (S1 and S2 stacked on partitions) --
            sp = ps_big.tile([2 * TLOC, NT], F32, tag="big")
            for c in range(4):
                nc.tensor.matmul(
                    sp[:], q12t_sb[:, c, :], kt_sb[:, c, :],
                    start=(c == 0), stop=(c == 3),
                )

            # ---------------- masked gumbel softmax ----------------
            dve_gate(a2_sb[0:1, 0:1], b2_sb[0:1, 0:1], vad_sb[0:1, 0:1])
            w_sb = sb.tile([2 * TLOC, NT], F32, tag="w")
            nc.vector.tensor_scalar(w_sb[:], sp[:], -40.0, 40.0, ALU.max, ALU.min)
            nc.vector.tensor_tensor(w_sb[:], w_sb[:], a2_sb[:], ALU.mult)
            nc.vector.tensor_tensor(w_sb[:], w_sb[:], b2_sb[:], ALU.add)
            mxn = sb.tile([2 * TLOC, 1], F32, tag="mxn")
            nc.vector.tensor_reduce(mxn[:], w_sb[:], axis=AX.X, op=ALU.max,
                                    negate=True)
            ex_sb = sb.tile([2 * TLOC, NT], F32, tag="ex")
            sm_sb = sb.tile([2 * TLOC, 1], F32, tag="sm1")
            nc.scalar.activation(ex_sb[:], w_sb[:], AF.Exp, bias=mxn[:],
                                 accum_out=sm_sb[:])
            rs_sb = sb.tile([2 * TLOC, 1], F32, tag="rs")
            nc.vector.reciprocal(rs_sb[:], sm_sb[:])
            att_sb = sb.tile([2 * TLOC, NT], F32, tag="att")
            nc.vector.tensor_scalar(att_sb[:], ex_sb[:], rs_sb[:], None, ALU.mult)
            nc.vector.tensor_tensor(att_sb[:], att_sb[:], vad_sb[:], ALU.add)

            # ---------------- att^T ----------------
            at_sb = sb.tile([128, 4, 2 * TLOC], F32, tag="at")
            for c in range(4):
                tr = ps_tr.tile([128, 2 * TLOC], F32, tag="tr")
                nc.tensor.transpose(
                    tr[:], att_sb[:, c * 128:(c + 1) * 128],
                    id_sb[:2 * TLOC, :2 * TLOC],
                )
                nc.vector.tensor_copy(at_sb[:, c, :], tr[:])

            # ---------------- e1^T, e2^T ----------------
            e1t_sb = sb.tile([128, 4, TLOC], F32, tag="e1t")
            e2t_sb = sb.tile([128, 4, TLOC], F32, tag="e2t")
            for ei, dst in ((0, e1t_sb), (1, e2t_sb)):
                for co in range(4):
                    ep = ps_sm.tile([128, TLOC], F32, tag="sm")
                    for cm in range(4):
                        nc.tensor.matmul(
                            ep[:],
                            ne_sb[:, cm, co * 128:(co + 1) * 128],
                            at_sb[:, cm, ei * TLOC:(ei + 1) * TLOC],
                            start=(cm == 0),
                            stop=(cm == 3),
                        )
                    nc.vector.tensor_copy(dst[:, co, :], ep[:])

            # ---------------- v1, v2 ----------------
            pe_gate(nvp_sb[0:1, 0:1])
            v_sb = []
            for ei in range(2):
                vp = ps_sm.tile([TLOC, 1], F32, tag="sm")
                for cm in range(4):
                    nc.tensor.matmul(
                        vp[:],
                        at_sb[:, cm, ei * TLOC:(ei + 1) * TLOC],
                        nvp_sb[:, cm:cm + 1],
                        start=(cm == 0),
                        stop=(cm == 3),
                    )
                vs = sb.tile([TLOC, 1], F32, tag=f"v{ei}s")
                nc.vector.tensor_copy(vs[:], vp[:])
                v_sb.append(vs)
            v1s, v2s = v_sb

            # ---------------- op selector: logits, gumbel-hard weights ------
            pe_gate(wsel_sb[0:1, 0, 0:1], ones_sb[0:1, 0:1], bsel_sb[0:1, 0:1])
            dve_gate(g3h_sb[0:1, 0:1])
            selp = ps_sm.tile([TLOC, NOPS], F32, tag="sm")
            for c in range(4):
                nc.tensor.matmul(selp[:], opqt_sb[:, c, :], wsel_sb[:, c, :],
                                 start=(c == 0), stop=False)
            nc.tensor.matmul(selp[:], ones_sb[0:1, 0:TLOC], bsel_sb[:],
                             start=False, stop=True)
            z3 = sb.tile([TLOC, NOPS], F32, tag="z3")
            nc.vector.tensor_scalar(z3[:], selp[:], -40.0, 40.0, ALU.max, ALU.min)
            w3 = sb.tile([TLOC, NOPS], F32, tag="w3")
            nc.vector.scalar_tensor_tensor(w3[:], z3[:], 0.5, g3h_sb[:],
                                           ALU.mult, ALU.add)
            mx3 = sb.tile([TLOC, 1], F32, tag="mx3")
            nc.vector.tensor_reduce(mx3[:], w3[:], axis=AX.X, op=ALU.max,
                                    negate=True)
            e3 = sb.tile([TLOC, NOPS], F32, tag="e3")
            s3 = sb.tile([TLOC, 1], F32, tag="s3")
            nc.scalar.activation(e3[:], w3[:], AF.Exp, bias=mx3[:], accum_out=s3[:])
            r3 = sb.tile([TLOC, 1], F32, tag="r3")
            nc.vector.reciprocal(r3[:], s3[:])
            y3 = sb.tile([TLOC, NOPS], F32, tag="y3")
            nc.vector.tensor_scalar(y3[:], e3[:], r3[:], None, ALU.mult)
            # hard one-hot: positions where w3 == max(w3)  (d = w3 - mx >= 0)
            d3 = sb.tile([TLOC, NOPS], F32, tag="d3")
            nc.vector.tensor_scalar(d3[:], w3[:], mx3[:], None, ALU.add)
            h3 = sb.tile([TLOC, NOPS], F32, tag="h3")
            nc.vector.tensor_scalar(h3[:], d3[:], 0.0, None, ALU.is_ge)
            # straight-through forward value: (h + y) - y + 1e-10 (exact fp order)
            opw = sb.tile([TLOC, NOPS], F32, tag="opw")
            nc.vector.tensor_tensor(opw[:], h3[:], y3[:], ALU.add)
            nc.vector.tensor_tensor(opw[:], opw[:], y3[:], ALU.subtract)
            nc.vector.tensor_scalar(opw[:], opw[:], 1e-10, None, ALU.add)

            # ---------------- the 9 elementwise ops on (v1, v2) ----------------
            outs = sb.tile([TLOC, NOPS], F32, tag="outs")
            nc.vector.tensor_tensor(outs[:, 0:1], v1s[:], v2s[:], ALU.add)
            nc.vector.tensor_copy(outs[:, 1:2], v1s[:])
            nc.vector.tensor_tensor(outs[:, 2:3], v1s[:], v2s[:], ALU.mult)
            nc.vector.tensor_tensor(outs[:, 3:4], v1s[:], v2s[:], ALU.subtract)
            # div = clip(v1 / (|v2| + eps))
            av2a = sb.tile([TLOC, 1], F32, tag="av2a")
            nc.scalar.activation(av2a[:], v2s[:], AF.Abs)
            av2 = sb.tile([TLOC, 1], F32, tag="av2")
            nc.vector.tensor_scalar(av2[:], av2a[:], EPS, None, ALU.add)
            rv2 = sb.tile([TLOC, 1], F32, tag="rv2")
            nc.vector.reciprocal(rv2[:], av2[:])
            dv = sb.tile([TLOC, 1], F32, tag="dv")
            nc.vector.tensor_tensor(dv[:], v1s[:], rv2[:], ALU.mult)
            nc.vector.tensor_scalar(outs[:, 4:5], dv[:], -1e6, 1e6, ALU.max, ALU.min)
            # pow = clip(xs**ys * sign(v1)); xs = clip(|v1|+eps, 1e-8, 1e3)
            av1a = sb.tile([TLOC, 1], F32, tag="av1a")
            nc.scalar.activation(av1a[:], v1s[:], AF.Abs)
            av1 = sb.tile([TLOC, 1], F32, tag="av1")
            nc.vector.tensor_scalar(av1[:], av1a[:], EPS, None, ALU.add)
            xs = sb.tile([TLOC, 1], F32, tag="xs")
            nc.vector.tensor_scalar(xs[:], av1[:], 1e-8, 1e3, ALU.max, ALU.min)
            ys = sb.tile([TLOC, 1], F32, tag="ys")
            nc.vector.tensor_scalar(ys[:], v2s[:], -6.0, 6.0, ALU.max, ALU.min)
            lgx = sb.tile([TLOC, 1], F32, tag="lgx")
            nc.scalar.activation(lgx[:], xs[:], AF.Ln)
            pr = sb.tile([TLOC, 1], F32, tag="pr")
            nc.vector.tensor_tensor(pr[:], ys[:], lgx[:], ALU.mult)
            nc.vector.tensor_scalar(pr[:], pr[:], -87.0, 87.0, ALU.max, ALU.min)
            exp_pr = sb.tile([TLOC, 1], F32, tag="exp_pr")
            nc.scalar.activation(exp_pr[:], pr[:], AF.Exp)
            sg = sb.tile([TLOC, 1], F32, tag="sg")
            nc.scalar.activation(sg[:], v1s[:], AF.Sign)
            pw = sb.tile([TLOC, 1], F32, tag="pw")
            nc.vector.tensor_tensor(pw[:], exp_pr[:], sg[:], ALU.mult)
            nc.vector.tensor_scalar(outs[:, 5:6], pw[:], -1e6, 1e6, ALU.max, ALU.min)
            # log = clip(ln(|v1| + eps), +-20)
            lgv = sb.tile([TLOC, 1], F32, tag="lgv")
            nc.scalar.activation(lgv[:], av1[:], AF.Ln)
            nc.vector.tensor_scalar(outs[:, 6:7], lgv[:], -20.0, 20.0,
                                    ALU.max, ALU.min)
            nc.vector.tensor_tensor(outs[:, 7:8], v1s[:], v2s[:], ALU.max)
            nc.vector.tensor_tensor(outs[:, 8:9], v1s[:], v2s[:], ALU.min)

            # ---------------- new_val = sum(op_w * outs) ----------------
            nv9 = sb.tile([TLOC, NOPS], F32, tag="nv9")
            nc.vector.tensor_tensor(nv9[:], opw[:], outs[:], ALU.mult)
            nval = sb.tile([TLOC, 1], F32, tag="nval")
            nc.vector.tensor_reduce(nval[:], nv9[:], axis=AX.X, op=ALU.add)
            nc.gpsimd.dma_start(out=oval[:], in_=nval[:])

            # ---------------- v^T, (v*Wse + bse)^T ----------------
            vt_sb = sb.tile([1, 2 * TLOC], F32, tag="vt")
            for ei, vs in ((0, v1s), (1, v2s)):
                tr = ps_tr.tile([1, TLOC], F32, tag="tr")
                nc.tensor.transpose(tr[:], vs[:], id_sb[:TLOC, :TLOC])
                nc.vector.tensor_copy(vt_sb[0:1, ei * TLOC:(ei + 1) * TLOC], tr[:])

            vet_sb = []
            pe_gate(wse_sb[0:1, 0:1], bse_sb[0:1, 0:1])
            for ei in range(2):
                vet = sb.tile([128, 4, TLOC], F32, tag=f"v{ei}et")
                for co in range(4):
                    vp = ps_sm.tile([128, TLOC], F32, tag="sm")
                    nc.tensor.matmul(
                        vp[:], wse_sb[0:1, co * 128:(co + 1) * 128],
                        vt_sb[0:1, ei * TLOC:(ei + 1) * TLOC],
                        start=True, stop=False,
                    )
                    nc.tensor.matmul(
                        vp[:], bse_sb[0:1, co * 128:(co + 1) * 128],
                        ones_sb[0:1, 0:TLOC],
                        start=False, stop=True,
                    )
                    nc.vector.tensor_copy(vet[:, co, :], vp[:])
                vet_sb.append(vet)
            v1et_sb, v2et_sb = vet_sb

            # ---------------- op_w^T ----------------
            opwt_sb = sb.tile([NOPS, TLOC], F32, tag="opwt")
            trw = ps_tr.tile([NOPS, TLOC], F32, tag="tr")
            nc.tensor.transpose(trw[:], opw[:], id_sb[:TLOC, :TLOC])
            nc.vector.tensor_copy(opwt_sb[:], trw[:])

            # ---------------- new_emb = concat @ Wmlp' + bmlp ----------------
            # (0.5*(e1+e2) is folded into Wmlp' rows [0:1024] host-side)
            pe_gate(wmlp_sb[0:1, 0, 0:1], wmlpt_sb[0:1, 0:1], bmlp_sb[0:1, 0:1])
            mp = ps_mlp.tile([TLOC, H], F32, tag="mlp")
            first = True
            for base, lhs in ((0, e1t_sb), (4, e2t_sb), (8, v1et_sb), (12, v2et_sb)):
                for c in range(4):
                    nc.tensor.matmul(mp[:], lhs[:, c, :], wmlp_sb[:, base + c, :],
                                     start=first, stop=False)
                    first = False
            nc.tensor.matmul(mp[:], opwt_sb[:], wmlpt_sb[:], start=False, stop=False)
            nc.tensor.matmul(mp[:], ones_sb[0:1, 0:TLOC], bmlp_sb[:],
                             start=False, stop=True)
            oemb_sb = sb.tile([TLOC, H], F32, tag="oemb")
            nc.vector.tensor_copy(oemb_sb[:], mp[:])
            nc.gpsimd.dma_start(out=oemb[:], in_=oemb_sb[:])

    _CACHE["nc"] = nc
    return nc


# --------------------------------------------------------------------------
# Host wrapper
# --------------------------------------------------------------------------

def _in_maps(inputs):
    f32 = np.float32
    g1, g2, g3 = _gumbel()
    _, _, _, A, vadd = _mask_tables()

    node_embeds = np.asarray(inputs["node_embeds"], f32)
    node_values = np.asarray(inputs["node_values"], f32)
    operand_ctx = np.asarray(inputs["operand_ctx"], f32)
    op_ctx = np.asarray(inputs["op_ctx"], f32)

    wmlp = np.asarray(inputs["Wmlp"], f32).copy()
    eye = np.eye(H, dtype=f32) * f32(0.5)
    wmlp[0:H] += eye
    wmlp[H:2 * H] += eye

    shared = {
        "wq1": np.ascontiguousarray(inputs["Wq1"], f32),
        "wq2": np.ascontiguousarray(inputs["Wq2"], f32),
        "wopq": np.ascontiguousarray(inputs["Wopq"], f32),
        "wk": np.ascontiguousarray(inputs["Wk"], f32),
        "bq1t": _t128(np.asarray(inputs["bq1"], f32)),
        "bq2t": _t128(np.asarray(inputs["bq2"], f32)),
        "bopqt": _t128(np.asarray(inputs["bopq"], f32)),
        "bkt": _t128(np.asarray(inputs["bk"], f32)),
        "wsel": np.ascontiguousarray(inputs["Wsel"], f32),
        "bsel": np.asarray(inputs["bsel"], f32).reshape(1, NOPS),
        "wse": np.asarray(inputs["Wse"], f32).reshape(1, H),
        "bse": np.asarray(inputs["bse"], f32).reshape(1, H),
        "wmlp": wmlp,
        "bmlp": np.asarray(inputs["bmlp"], f32).reshape(1, H),
        "s0p": _t128(np.asarray(inputs["step0"], f32)),
        "ident": np.eye(128, dtype=f32),
        "ones": np.ones((1, 64), f32),
    }

    maps = []
    for core in range(NCORES):
        b, half = core // 2, core % 2
        ts = slice(half * TLOC, (half + 1) * TLOC)
        b1 = _b_const(g1, b)[ts]
        b2c = _b_const(g2, b)[ts]
        m = dict(shared)
        m.update({
            "ne": np.ascontiguousarray(node_embeds[b].reshape(NT, H)),
            "nvp": np.ascontiguousarray(
                node_values[b].reshape(NT).reshape(4, 128).T),
            "octx": np.ascontiguousarray(operand_ctx[b, ts]),
            "opctx": np.ascontiguousarray(op_ctx[b, ts]),
            "a2": np.ascontiguousarray(np.concatenate([A[ts], A[ts]], axis=0)),
            "b2": np.ascontiguousarray(np.concatenate([b1, b2c], axis=0)),
            "vad": np.ascontiguousarray(
                np.concatenate([vadd[ts], vadd[ts]], axis=0)),
            "g3h": np.ascontiguousarray(g3[b, ts] * f32(0.5)),
        })
        maps.append(m)
    return maps


def run(inputs, trace=False):
    nc = _build_nc()
    maps = _in_maps(inputs)
    try:
        res = run_bass_kernel_spmd(nc, maps, core_ids=list(range(NCORES)),
                                   trace=trace)
    except ModuleNotFoundError:
        # NTFF trace hook unavailable in this client; run without tracing.
        res = run_bass_kernel_spmd(nc, maps, core_ids=list(range(NCORES)),
                                   trace=False)
    f32 = np.float32
    emb = np.zeros((B, T, H), f32)
    val = np.zeros((B, T), f32)
    for core in range(NCORES):
        b, half = core // 2, core % 2
        ts = slice(half * TLOC, (half + 1) * TLOC)
        emb[b, ts] = res.results[core]["oemb"]
        val[b, ts] = res.results[core]["oval"][:, 0]
    node_embeds = np.asarray(inputs["node_embeds"], f32)
    node_values = np.asarray(inputs["node_values"], f32)
    new_embeds = np.concatenate([node_embeds, emb[:, None]], axis=1)
    new_values = np.concatenate([node_values, val[:, None]], axis=1)
    return (new_embeds, new_values), res


def kernel(**inputs):
    return run(inputs, trace=False)[0]
